# revision 52
# baseline (speedup 1.0000x reference)
"""Distributed Bass kernel for attention (B=4,S=1024,D=4096,H=32,HD=128).

Sharding: 8 cores = 4 batch x 2 head-groups of 16 heads (core c: batch c//2,
heads (c%2)*16..+16). Per-core pipeline (all matmuls bf16 with fp32 PSUM):

  1. QKV projections. q/k computed feature-major (q^T[hd,tok] per head);
     2 heads x full token range per pass so weights are read exactly once,
     2 dc chunks per weight DMA. Partial rotary (block-order trick) on DVE,
     q/k bounce via DRAM.
  2. Attention per (head, query-half) with the host-precomputed exp(bias)
     factorization: E = exp(s) * expB (exp straight from PSUM, bf16 DVE
     multiply). Denominator via ones-vector matmul; 1/s broadcast with a
     bf16 rank-1 matmul.
  3. Output projection in (m-512-block, token-half) chunks with wo_bias/2
     folded in as a rank-1 matmul; pairwise ReduceScatter per chunk; bf16
     output written by deferred plain copies (host casts to fp32 - the RS
     result is already bf16 so this loses no precision).

Engine queues are FIFO, so phases are statically interleaved at emission
time: the PE-dense V pass is zipped with the ACT-paced qp=0 attention, and
qp=1 attention is zipped with the tg=0 output-projection chunks, keeping
the PE dense while ACT paces the softmax. PSUM banks are partitioned:
attention owns b0-b3 while the zipped partner owns b4-b7.
"""

import sys

sys.path.insert(0, "/opt/trn_rl_repo")

import numpy as np
import ml_dtypes

BF16 = ml_dtypes.bfloat16

B, S, D, H, HD = 4, 1024, 4096, 32, 128
ROTARY = 32
MAX_POS = 10000
HG = H // 2  # heads per core = 16
F = HG * HD  # per-core qkv feature dim = 2048
NCORES = 8
SCALE = 1.0 / np.sqrt(HD)
NDC = D // 128  # 32 contraction chunks
NKC = S // 128  # 8 key chunks

_cache = {}


def _build():
    import concourse.mybir as mybir
    import concourse.tile as tile
    from concourse import bacc

    fp32 = mybir.dt.float32
    bf16 = mybir.dt.bfloat16
    Act = mybir.ActivationFunctionType

    nc = bacc.Bacc("TRN2", target_bir_lowering=False, num_devices=NCORES)

    # ---- DRAM parameters (per-core shards) ----
    xT = nc.dram_tensor("xT", [D, S], bf16, kind="ExternalInput")
    wq = nc.dram_tensor("wq", [D, F], bf16, kind="ExternalInput")
    wk = nc.dram_tensor("wk", [D, F], bf16, kind="ExternalInput")
    wv = nc.dram_tensor("wv", [D, F], bf16, kind="ExternalInput")
    wo = nc.dram_tensor("wo", [F, D], bf16, kind="ExternalInput")
    bqT = nc.dram_tensor("bqT", [HD, HG], fp32, kind="ExternalInput")
    bkT = nc.dram_tensor("bkT", [HD, HG], fp32, kind="ExternalInput")
    bo_bc = nc.dram_tensor("bo_bc", [128, D], bf16, kind="ExternalInput")
    expBT = nc.dram_tensor("expBT", [S, S], bf16, kind="ExternalInput")
    rotC = nc.dram_tensor("rotC", [16, S], bf16, kind="ExternalInput")
    rotS = nc.dram_tensor("rotS", [16, S], bf16, kind="ExternalInput")
    ones = nc.dram_tensor("ones", [128, 1], bf16, kind="ExternalInput")
    ones_row_bf = nc.dram_tensor("ones_row_bf", [1, 128], bf16, kind="ExternalInput")
    out = nc.dram_tensor("out", [4, 2, 2, 128, 1024], bf16, kind="ExternalOutput")

    RG = [[0, 1], [2, 3], [4, 5], [6, 7]]

    with tile.TileContext(nc) as tc:
        with (
            tc.tile_pool(name="wpool", bufs=2) as wpool,
            tc.tile_pool(name="wvpool", bufs=2) as wvpool,
            tc.tile_pool(name="stream", bufs=2) as stream,
            tc.tile_pool(name="stage", bufs=2) as stpool,
            tc.tile_pool(name="tmp", bufs=1) as tmppool,
            tc.tile_pool(name="small", bufs=1) as small,
            tc.tile_pool(name="epool", bufs=1) as epool,
            tc.tile_pool(name="big", bufs=1) as big,
            tc.tile_pool(name="evac", bufs=2) as evacpool,
            tc.tile_pool(name="outp", bufs=2) as outpool,
            tc.tile_pool(name="ps", bufs=1, space="PSUM") as pspool,
            tc.tile_pool(name="dram", bufs=1, space="DRAM") as dram,
        ):
            # ---- constants ----
            bqT_sb = small.tile([HD, HG], fp32)
            bkT_sb = small.tile([HD, HG], fp32)
            bo_sb = small.tile([128, D], bf16)
            rotC_sb = small.tile([16, S], bf16)
            rotS_sb = small.tile([16, S], bf16)
            ones_sb = small.tile([128, 1], bf16)
            ones_row_bf_sb = small.tile([1, 128], bf16)
            expB_sb = small.tile([128, NKC, S], bf16)
            nc.sync.dma_start(ones_row_bf_sb[:], ones_row_bf[:])
            nc.sync.dma_start(bqT_sb[:], bqT[:])
            nc.sync.dma_start(bkT_sb[:], bkT[:])
            nc.sync.dma_start(bo_sb[:], bo_bc[:])
            nc.sync.dma_start(rotC_sb[:], rotC[:])
            nc.sync.dma_start(rotS_sb[:], rotS[:])
            nc.sync.dma_start(ones_sb[:], ones[:])
            nc.scalar.dma_start(
                expB_sb[:], expBT[:].rearrange("(kc p) q -> p kc q", p=128)
            )

            # resident input activations [d, dc, tok] (64KB/part);
            # oT_sb later reuses this slot (x is dead after the V pass)
            xT_sb = big.tile([128, NDC, S], bf16, tag="bigbuf", name="xT_sb")
            for i in range(8):
                nc.sync.dma_start(
                    xT_sb[:, i * 4 : (i + 1) * 4, :],
                    xT[i * 512 : (i + 1) * 512, :].rearrange(
                        "(a p) t -> p a t", p=128
                    ),
                )

            # DRAM bounce tensors
            q_dram = dram.tile([HG, 128, S], bf16, name="q_dram")
            k_dram = dram.tile([HG, 128, S], bf16, name="k_dram")
            v_dram = dram.tile([NKC, 128, F], bf16, name="v_dram")

            # ================= Phase 1: q/k projections =================
            pass_idx = [0]

            def qk_pass(w_dram, bias_sb, dst_dram, which):
                for hg2 in range(8):
                    par = (pass_idx[0] % 2) * 4
                    pass_idx[0] += 1
                    ps = {
                        (hi, th): pspool.tile(
                            [128, 512], fp32, tag=f"b{par + hi * 2 + th}",
                            name=f"ps{which}{hi}{th}",
                        )
                        for hi in range(2)
                        for th in range(2)
                    }
                    qbfs = [
                        stpool.tile(
                            [128, S], bf16, tag="qbf", bufs=5, name=f"qbf{i}"
                        )
                        for i in range(2)
                    ]
                    for dc2 in range(16):
                        wt = wpool.tile(
                            [128, 2, 256], bf16, tag=f"w{which}", bufs=4,
                            name="wt",
                        )
                        nc.sync.dma_start(
                            wt[:],
                            w_dram[
                                dc2 * 256 : (dc2 + 1) * 256,
                                hg2 * 256 : (hg2 + 1) * 256,
                            ].rearrange("(a p) f -> p a f", p=128),
                        )
                        for a in range(2):
                            dc = dc2 * 2 + a
                            for hi in range(2):
                                for th in range(2):
                                    nc.tensor.matmul(
                                        ps[(hi, th)][:],
                                        wt[:, a, hi * 128 : (hi + 1) * 128],
                                        xT_sb[:, dc, th * 512 : (th + 1) * 512],
                                        start=(dc == 0),
                                        stop=(dc == NDC - 1),
                                    )
                    for hi in range(2):
                        h = hg2 * 2 + hi
                        qbf = qbfs[hi]
                        for th in range(2):
                            nc.scalar.activation(
                                qbf[:, th * 512 : (th + 1) * 512],
                                ps[(hi, th)][:],
                                Act.Identity,
                                bias=bias_sb[:, h : h + 1],
                            )
                        # rotary (block order): u=qbf[0:16], w=qbf[16:32].
                        # Engine ops need 32-aligned partition bases, so the
                        # w half bounces via DMA to a base-0 tile and the f
                        # result bounces back to partitions 16:32.
                        rot_w = tmppool.tile([16, S], bf16, tag="rw", name="rot_w")
                        nc.scalar.dma_start(rot_w[:], qbf[16:32, :])
                        t1 = tmppool.tile([16, S], bf16, tag="t1", name="t1")
                        t2 = tmppool.tile([16, S], bf16, tag="t2", name="t2")
                        t3 = tmppool.tile([16, S], bf16, tag="t3", name="t3")
                        t4 = tmppool.tile([16, S], bf16, tag="t4", name="t4")
                        fbuf = tmppool.tile([16, S], bf16, tag="fb", name="fbuf")
                        u = qbf[0:16, :]
                        nc.vector.tensor_mul(t1[:], u, rotC_sb[:])
                        nc.vector.tensor_mul(t2[:], u, rotS_sb[:])
                        nc.vector.tensor_mul(t3[:], rot_w[:], rotS_sb[:])
                        nc.vector.tensor_mul(t4[:], rot_w[:], rotC_sb[:])
                        nc.vector.tensor_sub(qbf[0:16, :], t1[:], t3[:])
                        nc.vector.tensor_add(fbuf[:], t2[:], t4[:])
                        nc.gpsimd.dma_start(qbf[16:32, :], fbuf[:])
                        nc.gpsimd.dma_start(dst_dram[h], qbf[:])

            qk_pass(wq, bqT_sb, q_dram, "q")
            qk_pass(wk, bkT_sb, k_dram, "k")

            oT_sb = big.tile(
                [128, HG, S], bf16, tag="bigbuf", name="oT_sb"
            )  # [hd, h, tok]
            fins = []

            # ---- V pass generator (token-major), fixed banks b4..b7 ----
            def v_pass_gen():
                for fh2 in range(4):
                    f0 = fh2 * 512
                    for tq4 in range(2):
                        ps = {
                            ti: pspool.tile(
                                [128, 512], fp32, tag=f"b{4 + ti}",
                                name=f"psv{ti}",
                            )
                            for ti in range(4)
                        }
                        for dc2 in range(16):
                            wt = wvpool.tile(
                                [128, 2, 512], bf16, tag="wvo", bufs=8,
                                name="wvt",
                            )
                            nc.sync.dma_start(
                                wt[:],
                                wv[
                                    dc2 * 256 : (dc2 + 1) * 256, f0 : f0 + 512
                                ].rearrange("(a p) f -> p a f", p=128),
                            )
                            for a in range(2):
                                dc = dc2 * 2 + a
                                for ti in range(4):
                                    tci = tq4 * 4 + ti
                                    nc.tensor.matmul(
                                        ps[ti][:],
                                        xT_sb[:, dc, tci * 128 : (tci + 1) * 128],
                                        wt[:, a, :],
                                        start=(dc == 0),
                                        stop=(dc == NDC - 1),
                                    )
                            yield
                        for ti in range(4):
                            tci = tq4 * 4 + ti
                            vb = evacpool.tile(
                                [128, 512], bf16, tag="vb", name="vb"
                            )
                            # wv_bias is folded into wo_bias host-side
                            # (softmax weights sum to 1), so evacuation is
                            # a pure copy - split across ACT and DVE for
                            # faster bank release
                            if ti % 2 == 0:
                                nc.scalar.activation(
                                    vb[:], ps[ti][:], Act.Copy
                                )
                            else:
                                nc.vector.tensor_copy(vb[:], ps[ti][:])
                            nc.scalar.dma_start(
                                v_dram[tci, :, f0 : f0 + 512], vb[:]
                            )
                            # fine-grained yields let zipped attention
                            # units fill the bank-evacuation bubble
                            yield

            # ---- attention generator: one (head, query-half), b0..b3 ----
            def attn_head(h, qp):
                q0 = qp * 512
                qh_t = stream.tile([128, 512], bf16, tag="qh", name="qh_t")
                kh_t = stream.tile([128, S], bf16, tag="kh", name="kh_t")
                vh_t = stream.tile([128, NKC, HD], bf16, tag="vh", name="vh_t")
                nc.sync.dma_start(qh_t[:], q_dram[h][:, q0 : q0 + 512])
                nc.sync.dma_start(kh_t[:], k_dram[h])
                nc.sync.dma_start(
                    vh_t[:],
                    v_dram[:, :, h * 128 : (h + 1) * 128].rearrange(
                        "kc p hd -> p kc hd"
                    ),
                )
                sum_ps = pspool.tile([1, 512], fp32, tag="b2", name="sum")
                o_ps = pspool.tile([128, 512], fp32, tag="b3", name="o")
                for half in range(2):
                    E_t = epool.tile(
                        [128, 4, 512], bf16, tag="E", bufs=3, name="E"
                    )
                    for kcl in range(4):
                        kc = half * 4 + kcl
                        sps = pspool.tile(
                            [128, 512], fp32, tag=f"b{kc % 2}", name="sps"
                        )
                        nc.tensor.matmul(
                            sps[:],
                            kh_t[:, kc * 128 : (kc + 1) * 128],
                            qh_t[:],
                            start=True,
                            stop=True,
                        )
                        tmp_e = tmppool.tile(
                            [128, 512], bf16, tag="te", bufs=6, name="tmp_e"
                        )
                        nc.scalar.activation(tmp_e[:], sps[:], Act.Exp)
                        nc.vector.tensor_mul(
                            E_t[:, kcl, :], tmp_e[:],
                            expB_sb[:, kc, q0 : q0 + 512],
                        )
                        nc.tensor.matmul(
                            sum_ps[:],
                            ones_sb[:],
                            E_t[:, kcl, :],
                            start=(kc == 0),
                            stop=(kc == NKC - 1),
                        )
                        nc.tensor.matmul(
                            o_ps[:],
                            vh_t[:, kc, :],
                            E_t[:, kcl, :],
                            start=(kc == 0),
                            stop=(kc == NKC - 1),
                        )
                        yield
                # normalize tail: broadcast the SUM with a rank-1 bf16
                # matmul (score slot, freed early), then reciprocal on all
                # 128 lanes at once - shortest serial chain.
                sum_bf = tmppool.tile(
                    [1, 512], bf16, tag="invbf", bufs=2, name="sum_bf"
                )
                nc.scalar.activation(sum_bf[:], sum_ps[:], Act.Copy)
                bc_ps = pspool.tile([128, 512], fp32, tag="b0", name="bc")
                nc.tensor.matmul(
                    bc_ps[:], ones_row_bf_sb[:], sum_bf[:],
                    start=True, stop=True,
                )
                inv_bc = tmppool.tile(
                    [128, 512], fp32, tag="bcsb", bufs=2, name="inv_bc"
                )
                nc.vector.reciprocal_approx_fast(inv_bc[:], bc_ps[:])
                nc.vector.tensor_mul(
                    oT_sb[:, h, q0 : q0 + 512], o_ps[:], inv_bc[:]
                )
                yield

            def attn_gen(qp, heads):
                for h in heads:
                    yield from attn_head(h, qp)

            # ---- out-projection chunk generator: (m-512-block, tg) ----
            rs_state = {}

            def outproj_chunk(mh8, tg, par, split):
                m0 = mh8 * 512
                if split:
                    # per-mh8 [512,512] RS: fires earlier, and the final
                    # collective on the kernel tail is half the size
                    rs_state["in"] = dram.tile(
                        [512, 512], bf16, tag="rsin2", bufs=2, name="rs_in2"
                    )
                    rs_state["out"] = dram.tile(
                        [256, 512], bf16, tag="rsout2", bufs=8, name="rs_out2"
                    )
                    mcol = 0
                elif mh8 % 2 == 0:
                    # one [512,1024] RS buffer per mh4 pair of compute
                    # chunks keeps the serial CC stream ahead of compute
                    rs_state["in"] = dram.tile(
                        [512, 1024], bf16, tag="rsin", bufs=2, name="rs_in"
                    )
                    rs_state["out"] = dram.tile(
                        [256, 1024], bf16, tag="rsout", bufs=8, name="rs_out"
                    )
                    mcol = 0
                else:
                    mcol = 512
                rs_in = rs_state["in"]
                rs_out = rs_state["out"]
                ps = {
                    ti: pspool.tile(
                        [128, 512], fp32, tag=f"b{par + ti}", name=f"pso{ti}"
                    )
                    for ti in range(4)
                }
                for cc2 in range(8):
                    wt = wvpool.tile(
                        [128, 2, 512], bf16, tag="wvo", bufs=8, name="wot"
                    )
                    nc.sync.dma_start(
                        wt[:],
                        wo[
                            cc2 * 256 : (cc2 + 1) * 256, m0 : m0 + 512
                        ].rearrange("(a p) m -> p a m", p=128),
                    )
                    for a in range(2):
                        cc = cc2 * 2 + a
                        for ti in range(4):
                            t128 = tg * 4 + ti
                            nc.tensor.matmul(
                                ps[ti][:],
                                oT_sb[:, cc, t128 * 128 : (t128 + 1) * 128],
                                wt[:, a, :],
                                start=(cc == 0),
                                stop=False,
                            )
                    yield
                for ti in range(4):
                    nc.tensor.matmul(
                        ps[ti][:],
                        ones_row_bf_sb[:],
                        bo_sb[0:1, m0 : m0 + 512],
                        start=False,
                        stop=True,
                    )
                for ti in range(4):
                    po = outpool.tile(
                        [128, 512], bf16, tag="po", bufs=4, name="po"
                    )
                    if ti % 2 == 0:
                        nc.scalar.activation(po[:], ps[ti][:], Act.Copy)
                    else:
                        nc.vector.tensor_copy(po[:], ps[ti][:])
                    nc.sync.dma_start(
                        rs_in[ti * 128 : (ti + 1) * 128, mcol : mcol + 512],
                        po[:],
                    )
                if split or mh8 % 2 == 1:
                    nc.gpsimd.collective_compute(
                        "ReduceScatter",
                        mybir.AluOpType.add,
                        replica_groups=RG,
                        ins=[rs_in[:].opt()],
                        outs=[rs_out[:].opt()],
                    )
                    fins.append((mh8, tg, rs_out, split))
                yield

            def outproj_gen(tg, alternate, split):
                for mh8 in range(8):
                    par = (mh8 % 2) * 4 if alternate else 4
                    yield from outproj_chunk(mh8, tg, par, split)

            def zip_gens(ga, gb, na, nb):
                done_a = done_b = False
                while not (done_a and done_b):
                    for _ in range(na):
                        if not done_a:
                            try:
                                next(ga)
                            except StopIteration:
                                done_a = True
                    for _ in range(nb):
                        if not done_b:
                            try:
                                next(gb)
                            except StopIteration:
                                done_b = True

            # V block fh2=0 solo (attention heads 0-3 need it), then zip
            # the rest of V with qp=0 attention for heads 0-11.
            vgen = v_pass_gen()
            for _ in range(48):
                next(vgen)
            zip_gens(attn_gen(0, range(12)), vgen, 1, 1)
            # heads 12-15 qp0 must FULLY precede the tg=0 chunks in
            # program order (Tile dependencies are program-order RAW;
            # a read emitted before its writer reads stale data)
            for _ in attn_gen(0, range(12, 16)):
                pass
            # qp=1 attention zipped with tg=0 out-projection
            zip_gens(attn_gen(1, range(16)), outproj_gen(0, False, False), 2, 1)
            # tg=1 out-projection: bank parity + per-mh8 split RS
            for _ in outproj_gen(1, True, True):
                pass

            # Deferred output copies (plain bf16) on the GPSIMD queue: an
            # RS-gated fin there can only delay the next collective
            # trigger, which waits on the serial CC stream anyway. On ACT/
            # Sync queues a hoisted fin head-of-line-blocks compute.
            for mh8, tg, rs_out, split in fins:
                for dh in range(2):
                    if split:
                        nc.gpsimd.dma_start(
                            out[mh8 // 2, tg, dh, :,
                                (mh8 % 2) * 512 : (mh8 % 2) * 512 + 512],
                            rs_out[dh * 128 : (dh + 1) * 128, :],
                        )
                    else:
                        nc.gpsimd.dma_start(
                            out[mh8 // 2, tg, dh, :, :],
                            rs_out[dh * 128 : (dh + 1) * 128, :],
                        )

    nc.finalize()
    return nc


def _prep_shards(x, attn_bias, wq_kernel, wq_bias, wk_kernel, wk_bias,
                 wv_kernel, wv_bias, wo_kernel, wo_bias):
    """Host-side shard prep. Returns in_maps (list of 8 dicts)."""
    freqs = 1.0 / 10000.0 ** (np.arange(0, ROTARY, 2) / ROTARY)  # [16]
    pos = np.arange(MAX_POS - S, MAX_POS)  # [S]
    ang = np.outer(freqs, pos)  # [16, S]
    rotC = np.cos(ang).astype(np.float32)
    rotS = np.sin(ang).astype(np.float32)
    ones = np.ones((128, 1), dtype=BF16)
    expBT = np.exp(np.ascontiguousarray(attn_bias[0, 0].T)).astype(BF16)

    in_maps = []
    for c in range(NCORES):
        b, g = c // 2, c % 2
        hs = slice(g * HG, (g + 1) * HG)
        in_maps.append(
            {
                "xT": np.ascontiguousarray(x[b].T).astype(BF16),
                "wq": (wq_kernel[:, hs, :].reshape(D, F) * SCALE).astype(BF16),
                "wk": wk_kernel[:, hs, :].reshape(D, F).astype(BF16),
                "wv": wv_kernel[:, hs, :].reshape(D, F).astype(BF16),
                "wo": wo_kernel[hs].reshape(F, D).astype(BF16),
                "bqT": np.ascontiguousarray((wq_bias[hs] * SCALE).T).astype(
                    np.float32
                ),
                "bkT": np.ascontiguousarray(wk_bias[hs].T).astype(np.float32),
                # wv_bias folded in exactly: softmax weights sum to 1, so
                # the v-bias contributes sum_h wv_bias_h . wo_h to every
                # output row - a constant vector addable to wo_bias.
                "bo_bc": np.broadcast_to(
                    (
                        wo_bias * 0.5
                        + np.einsum(
                            "hk,hkm->m",
                            wv_bias[hs].astype(np.float64),
                            wo_kernel[hs].astype(np.float64),
                        )
                    ).reshape(1, D),
                    (128, D),
                ).astype(BF16).copy(),
                "expBT": expBT,
                "rotC": rotC.astype(BF16),
                "rotS": rotS.astype(BF16),
                "ones": ones,
                "ones_row_bf": np.ones((1, 128), dtype=BF16),
            }
        )
    return in_maps


def kernel(x, attn_bias, wq_kernel, wq_bias, wk_kernel, wk_bias,
           wv_kernel, wv_bias, wo_kernel, wo_bias, _trace=False):
    from concourse import bass_utils

    if "nc" not in _cache:
        _cache["nc"] = _build()
    nc = _cache["nc"]

    in_maps = _prep_shards(
        np.asarray(x), np.asarray(attn_bias),
        np.asarray(wq_kernel), np.asarray(wq_bias),
        np.asarray(wk_kernel), np.asarray(wk_bias),
        np.asarray(wv_kernel), np.asarray(wv_bias),
        np.asarray(wo_kernel), np.asarray(wo_bias),
    )
    res = bass_utils.run_bass_kernel_spmd(
        nc, in_maps, core_ids=list(range(NCORES)), trace=_trace
    )
    _cache["last_results"] = res

    full = np.empty((B, S, D), dtype=np.float32)
    for b in range(B):
        lo = res.results[2 * b]["out"]  # [mh4, tg, 2, 128, 1024] bf16
        hi = res.results[2 * b + 1]["out"]
        for mh4 in range(4):
            ms = slice(mh4 * 1024, (mh4 + 1) * 1024)
            for tg in range(2):
                t0 = tg * 512
                full[b, t0 : t0 + 256, ms] = lo[mh4, tg].reshape(
                    256, 1024
                ).astype(np.float32)
                full[b, t0 + 256 : t0 + 512, ms] = hi[mh4, tg].reshape(
                    256, 1024
                ).astype(np.float32)
    return full


# revision 53
# speedup vs baseline: 1.0230x; 1.0230x over previous
"""Distributed Bass kernel for attention (B=4,S=1024,D=4096,H=32,HD=128).

Sharding: 8 cores = 4 batch x 2 head-groups of 16 heads (core c: batch c//2,
heads (c%2)*16..+16). Per-core pipeline (all matmuls bf16 with fp32 PSUM):

  1. QKV projections. q/k computed feature-major (q^T[hd,tok] per head);
     2 heads x full token range per pass so weights are read exactly once,
     2 dc chunks per weight DMA. Partial rotary (block-order trick) on DVE,
     q/k bounce via DRAM.
  2. Attention per (head, query-half) with the host-precomputed exp(bias)
     factorization: E = exp(s) * expB (exp straight from PSUM, bf16 DVE
     multiply). Denominator via ones-vector matmul; 1/s broadcast with a
     bf16 rank-1 matmul.
  3. Output projection in (m-512-block, token-half) chunks with wo_bias/2
     folded in as a rank-1 matmul; pairwise ReduceScatter per chunk; bf16
     output written by deferred plain copies (host casts to fp32 - the RS
     result is already bf16 so this loses no precision).

Engine queues are FIFO, so phases are statically interleaved at emission
time: the PE-dense V pass is zipped with the ACT-paced qp=0 attention, and
qp=1 attention is zipped with the tg=0 output-projection chunks, keeping
the PE dense while ACT paces the softmax. PSUM banks are partitioned:
attention owns b0-b3 while the zipped partner owns b4-b7.
"""

import sys

sys.path.insert(0, "/opt/trn_rl_repo")

import numpy as np
import ml_dtypes

BF16 = ml_dtypes.bfloat16

B, S, D, H, HD = 4, 1024, 4096, 32, 128
ROTARY = 32
MAX_POS = 10000
HG = H // 2  # heads per core = 16
F = HG * HD  # per-core qkv feature dim = 2048
NCORES = 8
SCALE = 1.0 / np.sqrt(HD)
NDC = D // 128  # 32 contraction chunks
NKC = S // 128  # 8 key chunks

_cache = {}


def _build():
    import concourse.mybir as mybir
    import concourse.tile as tile
    from concourse import bacc

    fp32 = mybir.dt.float32
    bf16 = mybir.dt.bfloat16
    Act = mybir.ActivationFunctionType

    nc = bacc.Bacc("TRN2", target_bir_lowering=False, num_devices=NCORES)

    # ---- DRAM parameters (per-core shards) ----
    xT = nc.dram_tensor("xT", [D, S], bf16, kind="ExternalInput")
    wq = nc.dram_tensor("wq", [D, F], bf16, kind="ExternalInput")
    wk = nc.dram_tensor("wk", [D, F], bf16, kind="ExternalInput")
    wv = nc.dram_tensor("wv", [D, F], bf16, kind="ExternalInput")
    wo = nc.dram_tensor("wo", [F, D], bf16, kind="ExternalInput")
    bqT = nc.dram_tensor("bqT", [HD, HG], fp32, kind="ExternalInput")
    bkT = nc.dram_tensor("bkT", [HD, HG], fp32, kind="ExternalInput")
    bo_bc = nc.dram_tensor("bo_bc", [128, D], bf16, kind="ExternalInput")
    expBT = nc.dram_tensor("expBT", [S, S], bf16, kind="ExternalInput")
    rotC = nc.dram_tensor("rotC", [16, S], bf16, kind="ExternalInput")
    rotS = nc.dram_tensor("rotS", [16, S], bf16, kind="ExternalInput")
    ones = nc.dram_tensor("ones", [128, 1], bf16, kind="ExternalInput")
    ones_row_bf = nc.dram_tensor("ones_row_bf", [1, 128], bf16, kind="ExternalInput")
    out = nc.dram_tensor("out", [4, 2, 2, 128, 1024], bf16, kind="ExternalOutput")

    RG = [[0, 1], [2, 3], [4, 5], [6, 7]]

    with tile.TileContext(nc) as tc:
        with (
            tc.tile_pool(name="wpool", bufs=2) as wpool,
            tc.tile_pool(name="wvpool", bufs=2) as wvpool,
            tc.tile_pool(name="stream", bufs=2) as stream,
            tc.tile_pool(name="stage", bufs=2) as stpool,
            tc.tile_pool(name="tmp", bufs=1) as tmppool,
            tc.tile_pool(name="small", bufs=1) as small,
            tc.tile_pool(name="epool", bufs=1) as epool,
            tc.tile_pool(name="big", bufs=1) as big,
            tc.tile_pool(name="evac", bufs=2) as evacpool,
            tc.tile_pool(name="outp", bufs=2) as outpool,
            tc.tile_pool(name="ps", bufs=1, space="PSUM") as pspool,
            tc.tile_pool(name="dram", bufs=1, space="DRAM") as dram,
        ):
            # ---- constants ----
            bqT_sb = small.tile([HD, HG], fp32)
            bkT_sb = small.tile([HD, HG], fp32)
            bo_sb = small.tile([128, D], bf16)
            rotC_sb = small.tile([16, S], bf16)
            rotS_sb = small.tile([16, S], bf16)
            ones_sb = small.tile([128, 1], bf16)
            ones_row_bf_sb = small.tile([1, 128], bf16)
            expB_sb = small.tile([128, NKC, S], bf16)
            nc.sync.dma_start(ones_row_bf_sb[:], ones_row_bf[:])
            nc.sync.dma_start(bqT_sb[:], bqT[:])
            nc.sync.dma_start(bkT_sb[:], bkT[:])
            nc.sync.dma_start(bo_sb[:], bo_bc[:])
            nc.sync.dma_start(rotC_sb[:], rotC[:])
            nc.sync.dma_start(rotS_sb[:], rotS[:])
            nc.sync.dma_start(ones_sb[:], ones[:])
            nc.scalar.dma_start(
                expB_sb[:], expBT[:].rearrange("(kc p) q -> p kc q", p=128)
            )

            # resident input activations [d, dc, tok] (64KB/part);
            # oT_sb later reuses this slot (x is dead after the V pass)
            xT_sb = big.tile([128, NDC, S], bf16, tag="bigbuf", name="xT_sb")
            for i in range(8):
                nc.sync.dma_start(
                    xT_sb[:, i * 4 : (i + 1) * 4, :],
                    xT[i * 512 : (i + 1) * 512, :].rearrange(
                        "(a p) t -> p a t", p=128
                    ),
                )

            # DRAM bounce tensors
            q_dram = dram.tile([HG, 128, S], bf16, name="q_dram")
            k_dram = dram.tile([HG, 128, S], bf16, name="k_dram")
            v_dram = dram.tile([NKC, 128, F], bf16, name="v_dram")

            # ================= Phase 1: q/k projections =================
            pass_idx = [0]

            def qk_pass(w_dram, bias_sb, dst_dram, which):
                for hg2 in range(8):
                    par = (pass_idx[0] % 2) * 4
                    pass_idx[0] += 1
                    ps = {
                        (hi, th): pspool.tile(
                            [128, 512], fp32, tag=f"b{par + hi * 2 + th}",
                            name=f"ps{which}{hi}{th}",
                        )
                        for hi in range(2)
                        for th in range(2)
                    }
                    qbfs = [
                        stpool.tile(
                            [128, S], bf16, tag="qbf", bufs=5, name=f"qbf{i}"
                        )
                        for i in range(2)
                    ]
                    for dc2 in range(16):
                        wt = wpool.tile(
                            [128, 2, 256], bf16, tag=f"w{which}", bufs=4,
                            name="wt",
                        )
                        nc.sync.dma_start(
                            wt[:],
                            w_dram[
                                dc2 * 256 : (dc2 + 1) * 256,
                                hg2 * 256 : (hg2 + 1) * 256,
                            ].rearrange("(a p) f -> p a f", p=128),
                        )
                        for a in range(2):
                            dc = dc2 * 2 + a
                            for hi in range(2):
                                for th in range(2):
                                    nc.tensor.matmul(
                                        ps[(hi, th)][:],
                                        wt[:, a, hi * 128 : (hi + 1) * 128],
                                        xT_sb[:, dc, th * 512 : (th + 1) * 512],
                                        start=(dc == 0),
                                        stop=(dc == NDC - 1),
                                    )
                    for hi in range(2):
                        h = hg2 * 2 + hi
                        qbf = qbfs[hi]
                        for th in range(2):
                            nc.scalar.activation(
                                qbf[:, th * 512 : (th + 1) * 512],
                                ps[(hi, th)][:],
                                Act.Identity,
                                bias=bias_sb[:, h : h + 1],
                            )
                        # rotary (block order): u=qbf[0:16], w=qbf[16:32].
                        # Engine ops need 32-aligned partition bases, so the
                        # w half bounces via DMA to a base-0 tile and the f
                        # result bounces back to partitions 16:32.
                        rot_w = tmppool.tile([16, S], bf16, tag="rw", name="rot_w")
                        nc.scalar.dma_start(rot_w[:], qbf[16:32, :])
                        t1 = tmppool.tile([16, S], bf16, tag="t1", name="t1")
                        t2 = tmppool.tile([16, S], bf16, tag="t2", name="t2")
                        t3 = tmppool.tile([16, S], bf16, tag="t3", name="t3")
                        t4 = tmppool.tile([16, S], bf16, tag="t4", name="t4")
                        fbuf = tmppool.tile([16, S], bf16, tag="fb", name="fbuf")
                        u = qbf[0:16, :]
                        nc.vector.tensor_mul(t1[:], u, rotC_sb[:])
                        nc.vector.tensor_mul(t2[:], u, rotS_sb[:])
                        nc.vector.tensor_mul(t3[:], rot_w[:], rotS_sb[:])
                        nc.vector.tensor_mul(t4[:], rot_w[:], rotC_sb[:])
                        nc.vector.tensor_sub(qbf[0:16, :], t1[:], t3[:])
                        nc.vector.tensor_add(fbuf[:], t2[:], t4[:])
                        nc.gpsimd.dma_start(qbf[16:32, :], fbuf[:])
                        nc.gpsimd.dma_start(dst_dram[h], qbf[:])

            qk_pass(wq, bqT_sb, q_dram, "q")
            qk_pass(wk, bkT_sb, k_dram, "k")

            oT_sb = big.tile(
                [128, HG, S], bf16, tag="bigbuf", name="oT_sb"
            )  # [hd, h, tok]
            fins = []

            # ---- V pass generator (token-major), fixed banks b4..b7 ----
            def v_pass_gen():
                for fh2 in range(4):
                    f0 = fh2 * 512
                    for tq4 in range(2):
                        ps = {
                            ti: pspool.tile(
                                [128, 512], fp32, tag=f"b{4 + ti}",
                                name=f"psv{ti}",
                            )
                            for ti in range(4)
                        }
                        for dc2 in range(16):
                            wt = wvpool.tile(
                                [128, 2, 512], bf16, tag="wvo", bufs=8,
                                name="wvt",
                            )
                            nc.sync.dma_start(
                                wt[:],
                                wv[
                                    dc2 * 256 : (dc2 + 1) * 256, f0 : f0 + 512
                                ].rearrange("(a p) f -> p a f", p=128),
                            )
                            for a in range(2):
                                dc = dc2 * 2 + a
                                for ti in range(4):
                                    tci = tq4 * 4 + ti
                                    nc.tensor.matmul(
                                        ps[ti][:],
                                        xT_sb[:, dc, tci * 128 : (tci + 1) * 128],
                                        wt[:, a, :],
                                        start=(dc == 0),
                                        stop=(dc == NDC - 1),
                                    )
                                yield
                        for ti in range(4):
                            tci = tq4 * 4 + ti
                            vb = evacpool.tile(
                                [128, 512], bf16, tag="vb", name="vb"
                            )
                            # wv_bias is folded into wo_bias host-side
                            # (softmax weights sum to 1), so evacuation is
                            # a pure copy - split across ACT and DVE for
                            # faster bank release
                            if ti % 2 == 0:
                                nc.scalar.activation(
                                    vb[:], ps[ti][:], Act.Copy
                                )
                            else:
                                nc.vector.tensor_copy(vb[:], ps[ti][:])
                            nc.scalar.dma_start(
                                v_dram[tci, :, f0 : f0 + 512], vb[:]
                            )
                            # fine-grained yields let zipped attention
                            # units fill the bank-evacuation bubble
                            yield

            # ---- attention generator: one (head, query-half), b0..b3 ----
            def attn_head(h, qp):
                q0 = qp * 512
                qh_t = stream.tile([128, 512], bf16, tag="qh", name="qh_t")
                kh_t = stream.tile([128, S], bf16, tag="kh", name="kh_t")
                vh_t = stream.tile([128, NKC, HD], bf16, tag="vh", name="vh_t")
                nc.sync.dma_start(qh_t[:], q_dram[h][:, q0 : q0 + 512])
                nc.sync.dma_start(kh_t[:], k_dram[h])
                nc.sync.dma_start(
                    vh_t[:],
                    v_dram[:, :, h * 128 : (h + 1) * 128].rearrange(
                        "kc p hd -> p kc hd"
                    ),
                )
                sum_ps = pspool.tile([1, 512], fp32, tag="b2", name="sum")
                o_ps = pspool.tile([128, 512], fp32, tag="b3", name="o")
                for half in range(2):
                    E_t = epool.tile(
                        [128, 4, 512], bf16, tag="E", bufs=3, name="E"
                    )
                    for kcl in range(4):
                        kc = half * 4 + kcl
                        sps = pspool.tile(
                            [128, 512], fp32, tag=f"b{kc % 2}", name="sps"
                        )
                        nc.tensor.matmul(
                            sps[:],
                            kh_t[:, kc * 128 : (kc + 1) * 128],
                            qh_t[:],
                            start=True,
                            stop=True,
                        )
                        tmp_e = tmppool.tile(
                            [128, 512], bf16, tag="te", bufs=6, name="tmp_e"
                        )
                        nc.scalar.activation(tmp_e[:], sps[:], Act.Exp)
                        nc.vector.tensor_mul(
                            E_t[:, kcl, :], tmp_e[:],
                            expB_sb[:, kc, q0 : q0 + 512],
                        )
                        nc.tensor.matmul(
                            sum_ps[:],
                            ones_sb[:],
                            E_t[:, kcl, :],
                            start=(kc == 0),
                            stop=(kc == NKC - 1),
                        )
                        nc.tensor.matmul(
                            o_ps[:],
                            vh_t[:, kc, :],
                            E_t[:, kcl, :],
                            start=(kc == 0),
                            stop=(kc == NKC - 1),
                        )
                        yield
                # normalize tail: broadcast the SUM with a rank-1 bf16
                # matmul (score slot, freed early), then reciprocal on all
                # 128 lanes at once - shortest serial chain.
                sum_bf = tmppool.tile(
                    [1, 512], bf16, tag="invbf", bufs=2, name="sum_bf"
                )
                nc.scalar.activation(sum_bf[:], sum_ps[:], Act.Copy)
                bc_ps = pspool.tile([128, 512], fp32, tag="b0", name="bc")
                nc.tensor.matmul(
                    bc_ps[:], ones_row_bf_sb[:], sum_bf[:],
                    start=True, stop=True,
                )
                inv_bc = tmppool.tile(
                    [128, 512], fp32, tag="bcsb", bufs=2, name="inv_bc"
                )
                nc.vector.reciprocal_approx_fast(inv_bc[:], bc_ps[:])
                nc.vector.tensor_mul(
                    oT_sb[:, h, q0 : q0 + 512], o_ps[:], inv_bc[:]
                )
                yield

            def attn_gen(qp, heads):
                for h in heads:
                    yield from attn_head(h, qp)

            # ---- out-projection chunk generator: (m-512-block, tg) ----
            rs_state = {}

            def outproj_chunk(mh8, tg, par, split):
                m0 = mh8 * 512
                if split:
                    # per-mh8 [512,512] RS: fires earlier, and the final
                    # collective on the kernel tail is half the size
                    rs_state["in"] = dram.tile(
                        [512, 512], bf16, tag="rsin2", bufs=2, name="rs_in2"
                    )
                    rs_state["out"] = dram.tile(
                        [256, 512], bf16, tag="rsout2", bufs=8, name="rs_out2"
                    )
                    mcol = 0
                elif mh8 % 2 == 0:
                    # one [512,1024] RS buffer per mh4 pair of compute
                    # chunks keeps the serial CC stream ahead of compute
                    rs_state["in"] = dram.tile(
                        [512, 1024], bf16, tag="rsin", bufs=2, name="rs_in"
                    )
                    rs_state["out"] = dram.tile(
                        [256, 1024], bf16, tag="rsout", bufs=8, name="rs_out"
                    )
                    mcol = 0
                else:
                    mcol = 512
                rs_in = rs_state["in"]
                rs_out = rs_state["out"]
                ps = {
                    ti: pspool.tile(
                        [128, 512], fp32, tag=f"b{par + ti}", name=f"pso{ti}"
                    )
                    for ti in range(4)
                }
                for cc2 in range(8):
                    wt = wvpool.tile(
                        [128, 2, 512], bf16, tag="wvo", bufs=8, name="wot"
                    )
                    nc.sync.dma_start(
                        wt[:],
                        wo[
                            cc2 * 256 : (cc2 + 1) * 256, m0 : m0 + 512
                        ].rearrange("(a p) m -> p a m", p=128),
                    )
                    for a in range(2):
                        cc = cc2 * 2 + a
                        for ti in range(4):
                            t128 = tg * 4 + ti
                            nc.tensor.matmul(
                                ps[ti][:],
                                oT_sb[:, cc, t128 * 128 : (t128 + 1) * 128],
                                wt[:, a, :],
                                start=(cc == 0),
                                stop=False,
                            )
                        yield
                for ti in range(4):
                    nc.tensor.matmul(
                        ps[ti][:],
                        ones_row_bf_sb[:],
                        bo_sb[0:1, m0 : m0 + 512],
                        start=False,
                        stop=True,
                    )
                for ti in range(4):
                    po = outpool.tile(
                        [128, 512], bf16, tag="po", bufs=4, name="po"
                    )
                    if ti % 2 == 0:
                        nc.scalar.activation(po[:], ps[ti][:], Act.Copy)
                    else:
                        nc.vector.tensor_copy(po[:], ps[ti][:])
                    nc.sync.dma_start(
                        rs_in[ti * 128 : (ti + 1) * 128, mcol : mcol + 512],
                        po[:],
                    )
                if split or mh8 % 2 == 1:
                    nc.gpsimd.collective_compute(
                        "ReduceScatter",
                        mybir.AluOpType.add,
                        replica_groups=RG,
                        ins=[rs_in[:].opt()],
                        outs=[rs_out[:].opt()],
                    )
                    fins.append((mh8, tg, rs_out, split))
                yield

            def outproj_gen(tg, alternate, split):
                for mh8 in range(8):
                    par = (mh8 % 2) * 4 if alternate else 4
                    yield from outproj_chunk(mh8, tg, par, split)

            def zip_gens(ga, gb, na, nb):
                done_a = done_b = False
                while not (done_a and done_b):
                    for _ in range(na):
                        if not done_a:
                            try:
                                next(ga)
                            except StopIteration:
                                done_a = True
                    for _ in range(nb):
                        if not done_b:
                            try:
                                next(gb)
                            except StopIteration:
                                done_b = True

            # V block fh2=0 solo (attention heads 0-3 need it), then zip
            # the rest of V with qp=0 attention for heads 0-11.
            vgen = v_pass_gen()
            for _ in range(80):
                next(vgen)
            zip_gens(attn_gen(0, range(12)), vgen, 1, 2)
            # heads 12-15 qp0 must FULLY precede the tg=0 chunks in
            # program order (Tile dependencies are program-order RAW;
            # a read emitted before its writer reads stale data)
            for _ in attn_gen(0, range(12, 16)):
                pass
            # qp=1 attention zipped with tg=0 out-projection
            zip_gens(attn_gen(1, range(16)), outproj_gen(0, False, False), 1, 1)
            # tg=1 out-projection: bank parity + per-mh8 split RS
            for _ in outproj_gen(1, True, True):
                pass

            # Deferred output copies (plain bf16) on the GPSIMD queue: an
            # RS-gated fin there can only delay the next collective
            # trigger, which waits on the serial CC stream anyway. On ACT/
            # Sync queues a hoisted fin head-of-line-blocks compute.
            for mh8, tg, rs_out, split in fins:
                for dh in range(2):
                    if split:
                        nc.gpsimd.dma_start(
                            out[mh8 // 2, tg, dh, :,
                                (mh8 % 2) * 512 : (mh8 % 2) * 512 + 512],
                            rs_out[dh * 128 : (dh + 1) * 128, :],
                        )
                    else:
                        nc.gpsimd.dma_start(
                            out[mh8 // 2, tg, dh, :, :],
                            rs_out[dh * 128 : (dh + 1) * 128, :],
                        )

    nc.finalize()
    return nc


def _prep_shards(x, attn_bias, wq_kernel, wq_bias, wk_kernel, wk_bias,
                 wv_kernel, wv_bias, wo_kernel, wo_bias):
    """Host-side shard prep. Returns in_maps (list of 8 dicts)."""
    freqs = 1.0 / 10000.0 ** (np.arange(0, ROTARY, 2) / ROTARY)  # [16]
    pos = np.arange(MAX_POS - S, MAX_POS)  # [S]
    ang = np.outer(freqs, pos)  # [16, S]
    rotC = np.cos(ang).astype(np.float32)
    rotS = np.sin(ang).astype(np.float32)
    ones = np.ones((128, 1), dtype=BF16)
    expBT = np.exp(np.ascontiguousarray(attn_bias[0, 0].T)).astype(BF16)

    in_maps = []
    for c in range(NCORES):
        b, g = c // 2, c % 2
        hs = slice(g * HG, (g + 1) * HG)
        in_maps.append(
            {
                "xT": np.ascontiguousarray(x[b].T).astype(BF16),
                "wq": (wq_kernel[:, hs, :].reshape(D, F) * SCALE).astype(BF16),
                "wk": wk_kernel[:, hs, :].reshape(D, F).astype(BF16),
                "wv": wv_kernel[:, hs, :].reshape(D, F).astype(BF16),
                "wo": wo_kernel[hs].reshape(F, D).astype(BF16),
                "bqT": np.ascontiguousarray((wq_bias[hs] * SCALE).T).astype(
                    np.float32
                ),
                "bkT": np.ascontiguousarray(wk_bias[hs].T).astype(np.float32),
                # wv_bias folded in exactly: softmax weights sum to 1, so
                # the v-bias contributes sum_h wv_bias_h . wo_h to every
                # output row - a constant vector addable to wo_bias.
                "bo_bc": np.broadcast_to(
                    (
                        wo_bias * 0.5
                        + np.einsum(
                            "hk,hkm->m",
                            wv_bias[hs].astype(np.float64),
                            wo_kernel[hs].astype(np.float64),
                        )
                    ).reshape(1, D),
                    (128, D),
                ).astype(BF16).copy(),
                "expBT": expBT,
                "rotC": rotC.astype(BF16),
                "rotS": rotS.astype(BF16),
                "ones": ones,
                "ones_row_bf": np.ones((1, 128), dtype=BF16),
            }
        )
    return in_maps


def kernel(x, attn_bias, wq_kernel, wq_bias, wk_kernel, wk_bias,
           wv_kernel, wv_bias, wo_kernel, wo_bias, _trace=False):
    from concourse import bass_utils

    if "nc" not in _cache:
        _cache["nc"] = _build()
    nc = _cache["nc"]

    in_maps = _prep_shards(
        np.asarray(x), np.asarray(attn_bias),
        np.asarray(wq_kernel), np.asarray(wq_bias),
        np.asarray(wk_kernel), np.asarray(wk_bias),
        np.asarray(wv_kernel), np.asarray(wv_bias),
        np.asarray(wo_kernel), np.asarray(wo_bias),
    )
    res = bass_utils.run_bass_kernel_spmd(
        nc, in_maps, core_ids=list(range(NCORES)), trace=_trace
    )
    _cache["last_results"] = res

    full = np.empty((B, S, D), dtype=np.float32)
    for b in range(B):
        lo = res.results[2 * b]["out"]  # [mh4, tg, 2, 128, 1024] bf16
        hi = res.results[2 * b + 1]["out"]
        for mh4 in range(4):
            ms = slice(mh4 * 1024, (mh4 + 1) * 1024)
            for tg in range(2):
                t0 = tg * 512
                full[b, t0 : t0 + 256, ms] = lo[mh4, tg].reshape(
                    256, 1024
                ).astype(np.float32)
                full[b, t0 + 256 : t0 + 512, ms] = hi[mh4, tg].reshape(
                    256, 1024
                ).astype(np.float32)
    return full


# revision 55
# speedup vs baseline: 1.0271x; 1.0040x over previous
"""Distributed Bass kernel for attention (B=4,S=1024,D=4096,H=32,HD=128).

Sharding: 8 cores = 4 batch x 2 head-groups of 16 heads (core c: batch c//2,
heads (c%2)*16..+16). Per-core pipeline (all matmuls bf16 with fp32 PSUM):

  1. QKV projections. q/k computed feature-major (q^T[hd,tok] per head);
     2 heads x full token range per pass so weights are read exactly once,
     2 dc chunks per weight DMA. Partial rotary (block-order trick) on DVE,
     q/k bounce via DRAM.
  2. Attention per (head, query-half) with the host-precomputed exp(bias)
     factorization: E = exp(s) * expB (exp straight from PSUM, bf16 DVE
     multiply). Denominator via ones-vector matmul; 1/s broadcast with a
     bf16 rank-1 matmul.
  3. Output projection in (m-512-block, token-half) chunks with wo_bias/2
     folded in as a rank-1 matmul; pairwise ReduceScatter per chunk; bf16
     output written by deferred plain copies (host casts to fp32 - the RS
     result is already bf16 so this loses no precision).

Engine queues are FIFO, so phases are statically interleaved at emission
time: the PE-dense V pass is zipped with the ACT-paced qp=0 attention, and
qp=1 attention is zipped with the tg=0 output-projection chunks, keeping
the PE dense while ACT paces the softmax. PSUM banks are partitioned:
attention owns b0-b3 while the zipped partner owns b4-b7.
"""

import sys

sys.path.insert(0, "/opt/trn_rl_repo")

import numpy as np
import ml_dtypes

BF16 = ml_dtypes.bfloat16

B, S, D, H, HD = 4, 1024, 4096, 32, 128
ROTARY = 32
MAX_POS = 10000
HG = H // 2  # heads per core = 16
F = HG * HD  # per-core qkv feature dim = 2048
NCORES = 8
SCALE = 1.0 / np.sqrt(HD)
NDC = D // 128  # 32 contraction chunks
NKC = S // 128  # 8 key chunks

_cache = {}


def _build():
    import concourse.mybir as mybir
    import concourse.tile as tile
    from concourse import bacc

    fp32 = mybir.dt.float32
    bf16 = mybir.dt.bfloat16
    Act = mybir.ActivationFunctionType

    nc = bacc.Bacc("TRN2", target_bir_lowering=False, num_devices=NCORES)

    # ---- DRAM parameters (per-core shards) ----
    xT = nc.dram_tensor("xT", [D, S], bf16, kind="ExternalInput")
    wq = nc.dram_tensor("wq", [D, F], bf16, kind="ExternalInput")
    wk = nc.dram_tensor("wk", [D, F], bf16, kind="ExternalInput")
    wv = nc.dram_tensor("wv", [D, F], bf16, kind="ExternalInput")
    wo = nc.dram_tensor("wo", [F, D], bf16, kind="ExternalInput")
    bqT = nc.dram_tensor("bqT", [HD, HG], fp32, kind="ExternalInput")
    bkT = nc.dram_tensor("bkT", [HD, HG], fp32, kind="ExternalInput")
    bo_bc = nc.dram_tensor("bo_bc", [128, D], bf16, kind="ExternalInput")
    expBT = nc.dram_tensor("expBT", [S, S], bf16, kind="ExternalInput")
    rotC = nc.dram_tensor("rotC", [16, S], bf16, kind="ExternalInput")
    rotS = nc.dram_tensor("rotS", [16, S], bf16, kind="ExternalInput")
    ones = nc.dram_tensor("ones", [128, 1], bf16, kind="ExternalInput")
    ones_row_bf = nc.dram_tensor("ones_row_bf", [1, 128], bf16, kind="ExternalInput")
    out = nc.dram_tensor("out", [4, 2, 2, 128, 1024], bf16, kind="ExternalOutput")

    RG = [[0, 1], [2, 3], [4, 5], [6, 7]]

    with tile.TileContext(nc) as tc:
        with (
            tc.tile_pool(name="wpool", bufs=2) as wpool,
            tc.tile_pool(name="wvpool", bufs=2) as wvpool,
            tc.tile_pool(name="stream", bufs=2) as stream,
            tc.tile_pool(name="stage", bufs=2) as stpool,
            tc.tile_pool(name="tmp", bufs=1) as tmppool,
            tc.tile_pool(name="small", bufs=1) as small,
            tc.tile_pool(name="epool", bufs=1) as epool,
            tc.tile_pool(name="big", bufs=1) as big,
            tc.tile_pool(name="evac", bufs=2) as evacpool,
            tc.tile_pool(name="outp", bufs=2) as outpool,
            tc.tile_pool(name="ps", bufs=1, space="PSUM") as pspool,
            tc.tile_pool(name="dram", bufs=1, space="DRAM") as dram,
        ):
            # ---- constants ----
            bqT_sb = small.tile([HD, HG], fp32)
            bkT_sb = small.tile([HD, HG], fp32)
            bo_sb = small.tile([128, D], bf16)
            rotC_sb = small.tile([16, S], bf16)
            rotS_sb = small.tile([16, S], bf16)
            ones_sb = small.tile([128, 1], bf16)
            ones_row_bf_sb = small.tile([1, 128], bf16)
            expB_sb = small.tile([128, NKC, S], bf16)
            nc.sync.dma_start(ones_row_bf_sb[:], ones_row_bf[:])
            nc.sync.dma_start(bqT_sb[:], bqT[:])
            nc.sync.dma_start(bkT_sb[:], bkT[:])
            nc.sync.dma_start(bo_sb[:], bo_bc[:])
            nc.sync.dma_start(rotC_sb[:], rotC[:])
            nc.sync.dma_start(rotS_sb[:], rotS[:])
            nc.sync.dma_start(ones_sb[:], ones[:])
            nc.scalar.dma_start(
                expB_sb[:], expBT[:].rearrange("(kc p) q -> p kc q", p=128)
            )

            # resident input activations [d, dc, tok] (64KB/part);
            # oT_sb later reuses this slot (x is dead after the V pass)
            xT_sb = big.tile([128, NDC, S], bf16, tag="bigbuf", name="xT_sb")
            for i in range(8):
                nc.sync.dma_start(
                    xT_sb[:, i * 4 : (i + 1) * 4, :],
                    xT[i * 512 : (i + 1) * 512, :].rearrange(
                        "(a p) t -> p a t", p=128
                    ),
                )

            # DRAM bounce tensors
            q_dram = dram.tile([HG, 128, S], bf16, name="q_dram")
            k_dram = dram.tile([HG, 128, S], bf16, name="k_dram")
            v_dram = dram.tile([NKC, 128, F], bf16, name="v_dram")

            # ================= Phase 1: q/k projections =================
            pass_idx = [0]

            def qk_pass(w_dram, bias_sb, dst_dram, which):
                for hg2 in range(8):
                    par = (pass_idx[0] % 2) * 4
                    pass_idx[0] += 1
                    ps = {
                        (hi, th): pspool.tile(
                            [128, 512], fp32, tag=f"b{par + hi * 2 + th}",
                            name=f"ps{which}{hi}{th}",
                        )
                        for hi in range(2)
                        for th in range(2)
                    }
                    qbfs = [
                        stpool.tile(
                            [128, S], bf16, tag="qbf", bufs=5, name=f"qbf{i}"
                        )
                        for i in range(2)
                    ]
                    for dc2 in range(16):
                        wt = wpool.tile(
                            [128, 2, 256], bf16, tag=f"w{which}", bufs=4,
                            name="wt",
                        )
                        nc.sync.dma_start(
                            wt[:],
                            w_dram[
                                dc2 * 256 : (dc2 + 1) * 256,
                                hg2 * 256 : (hg2 + 1) * 256,
                            ].rearrange("(a p) f -> p a f", p=128),
                        )
                        for a in range(2):
                            dc = dc2 * 2 + a
                            for hi in range(2):
                                for th in range(2):
                                    nc.tensor.matmul(
                                        ps[(hi, th)][:],
                                        wt[:, a, hi * 128 : (hi + 1) * 128],
                                        xT_sb[:, dc, th * 512 : (th + 1) * 512],
                                        start=(dc == 0),
                                        stop=(dc == NDC - 1),
                                    )
                    for hi in range(2):
                        h = hg2 * 2 + hi
                        qbf = qbfs[hi]
                        for th in range(2):
                            nc.scalar.activation(
                                qbf[:, th * 512 : (th + 1) * 512],
                                ps[(hi, th)][:],
                                Act.Identity,
                                bias=bias_sb[:, h : h + 1],
                            )
                        # rotary (block order): u=qbf[0:16], w=qbf[16:32].
                        # Engine ops need 32-aligned partition bases, so the
                        # w half bounces via DMA to a base-0 tile and the f
                        # result bounces back to partitions 16:32.
                        rot_w = tmppool.tile([16, S], bf16, tag="rw", name="rot_w")
                        nc.scalar.dma_start(rot_w[:], qbf[16:32, :])
                        t1 = tmppool.tile([16, S], bf16, tag="t1", name="t1")
                        t2 = tmppool.tile([16, S], bf16, tag="t2", name="t2")
                        t3 = tmppool.tile([16, S], bf16, tag="t3", name="t3")
                        t4 = tmppool.tile([16, S], bf16, tag="t4", name="t4")
                        fbuf = tmppool.tile([16, S], bf16, tag="fb", name="fbuf")
                        u = qbf[0:16, :]
                        nc.vector.tensor_mul(t1[:], u, rotC_sb[:])
                        nc.vector.tensor_mul(t2[:], u, rotS_sb[:])
                        nc.vector.tensor_mul(t3[:], rot_w[:], rotS_sb[:])
                        nc.vector.tensor_mul(t4[:], rot_w[:], rotC_sb[:])
                        nc.vector.tensor_sub(qbf[0:16, :], t1[:], t3[:])
                        nc.vector.tensor_add(fbuf[:], t2[:], t4[:])
                        nc.gpsimd.dma_start(qbf[16:32, :], fbuf[:])
                        nc.gpsimd.dma_start(dst_dram[h], qbf[:])

            qk_pass(wq, bqT_sb, q_dram, "q")
            qk_pass(wk, bkT_sb, k_dram, "k")

            oT_sb = big.tile(
                [128, HG, S], bf16, tag="bigbuf", name="oT_sb"
            )  # [hd, h, tok]
            fins = []

            # ---- V pass generator (token-major), fixed banks b4..b7 ----
            def v_pass_gen():
                for fh2 in range(4):
                    f0 = fh2 * 512
                    for tq4 in range(2):
                        ps = {
                            ti: pspool.tile(
                                [128, 512], fp32, tag=f"b{4 + ti}",
                                name=f"psv{ti}",
                            )
                            for ti in range(4)
                        }
                        for dc2 in range(16):
                            wt = wvpool.tile(
                                [128, 2, 512], bf16, tag="wvo", bufs=8,
                                name="wvt",
                            )
                            nc.sync.dma_start(
                                wt[:],
                                wv[
                                    dc2 * 256 : (dc2 + 1) * 256, f0 : f0 + 512
                                ].rearrange("(a p) f -> p a f", p=128),
                            )
                            for a in range(2):
                                dc = dc2 * 2 + a
                                for ti in range(4):
                                    tci = tq4 * 4 + ti
                                    nc.tensor.matmul(
                                        ps[ti][:],
                                        xT_sb[:, dc, tci * 128 : (tci + 1) * 128],
                                        wt[:, a, :],
                                        start=(dc == 0),
                                        stop=(dc == NDC - 1),
                                    )
                            yield
                        for ti in range(4):
                            tci = tq4 * 4 + ti
                            vb = evacpool.tile(
                                [128, 512], bf16, tag="vb", bufs=4, name="vb"
                            )
                            # wv_bias is folded into wo_bias host-side
                            # (softmax weights sum to 1), so evacuation is
                            # a pure copy - split across ACT and DVE for
                            # faster bank release
                            if ti % 2 == 0:
                                nc.scalar.activation(
                                    vb[:], ps[ti][:], Act.Copy
                                )
                            else:
                                nc.vector.tensor_copy(vb[:], ps[ti][:])
                            nc.scalar.dma_start(
                                v_dram[tci, :, f0 : f0 + 512], vb[:]
                            )
                            # fine-grained yields let zipped attention
                            # units fill the bank-evacuation bubble
                            yield

            # ---- attention generator: one (head, query-half), b0..b3 ----
            def attn_head(h, qp):
                q0 = qp * 512
                qh_t = stream.tile([128, 512], bf16, tag="qh", name="qh_t")
                kh_t = stream.tile([128, S], bf16, tag="kh", name="kh_t")
                vh_t = stream.tile([128, NKC, HD], bf16, tag="vh", name="vh_t")
                nc.sync.dma_start(qh_t[:], q_dram[h][:, q0 : q0 + 512])
                nc.sync.dma_start(kh_t[:], k_dram[h])
                nc.sync.dma_start(
                    vh_t[:],
                    v_dram[:, :, h * 128 : (h + 1) * 128].rearrange(
                        "kc p hd -> p kc hd"
                    ),
                )
                sum_ps = pspool.tile([1, 512], fp32, tag="b2", name="sum")
                o_ps = pspool.tile([128, 512], fp32, tag="b3", name="o")
                for half in range(2):
                    E_t = epool.tile(
                        [128, 4, 512], bf16, tag="E", bufs=3, name="E"
                    )
                    for kcl in range(4):
                        kc = half * 4 + kcl
                        sps = pspool.tile(
                            [128, 512], fp32, tag=f"b{kc % 2}", name="sps"
                        )
                        nc.tensor.matmul(
                            sps[:],
                            kh_t[:, kc * 128 : (kc + 1) * 128],
                            qh_t[:],
                            start=True,
                            stop=True,
                        )
                        tmp_e = tmppool.tile(
                            [128, 512], bf16, tag="te", bufs=6, name="tmp_e"
                        )
                        nc.scalar.activation(tmp_e[:], sps[:], Act.Exp)
                        nc.vector.tensor_mul(
                            E_t[:, kcl, :], tmp_e[:],
                            expB_sb[:, kc, q0 : q0 + 512],
                        )
                        nc.tensor.matmul(
                            sum_ps[:],
                            ones_sb[:],
                            E_t[:, kcl, :],
                            start=(kc == 0),
                            stop=(kc == NKC - 1),
                        )
                        nc.tensor.matmul(
                            o_ps[:],
                            vh_t[:, kc, :],
                            E_t[:, kcl, :],
                            start=(kc == 0),
                            stop=(kc == NKC - 1),
                        )
                        yield
                # normalize tail: broadcast the SUM with a rank-1 bf16
                # matmul (score slot, freed early), then reciprocal on all
                # 128 lanes at once - shortest serial chain.
                sum_bf = tmppool.tile(
                    [1, 512], bf16, tag="invbf", bufs=2, name="sum_bf"
                )
                nc.scalar.activation(sum_bf[:], sum_ps[:], Act.Copy)
                bc_ps = pspool.tile([128, 512], fp32, tag="b0", name="bc")
                nc.tensor.matmul(
                    bc_ps[:], ones_row_bf_sb[:], sum_bf[:],
                    start=True, stop=True,
                )
                inv_bc = tmppool.tile(
                    [128, 512], fp32, tag="bcsb", bufs=2, name="inv_bc"
                )
                nc.vector.reciprocal_approx_fast(inv_bc[:], bc_ps[:])
                nc.vector.tensor_mul(
                    oT_sb[:, h, q0 : q0 + 512], o_ps[:], inv_bc[:]
                )
                yield

            def attn_gen(qp, heads):
                for h in heads:
                    yield from attn_head(h, qp)

            # ---- out-projection chunk generator: (m-512-block, tg) ----
            rs_state = {}

            def outproj_chunk(mh8, tg, par, split):
                m0 = mh8 * 512
                if split:
                    # per-mh8 [512,512] RS: fires earlier, and the final
                    # collective on the kernel tail is half the size
                    rs_state["in"] = dram.tile(
                        [512, 512], bf16, tag="rsin2", bufs=2, name="rs_in2"
                    )
                    rs_state["out"] = dram.tile(
                        [256, 512], bf16, tag="rsout2", bufs=8, name="rs_out2"
                    )
                    mcol = 0
                elif mh8 % 2 == 0:
                    # one [512,1024] RS buffer per mh4 pair of compute
                    # chunks keeps the serial CC stream ahead of compute
                    rs_state["in"] = dram.tile(
                        [512, 1024], bf16, tag="rsin", bufs=2, name="rs_in"
                    )
                    rs_state["out"] = dram.tile(
                        [256, 1024], bf16, tag="rsout", bufs=8, name="rs_out"
                    )
                    mcol = 0
                else:
                    mcol = 512
                rs_in = rs_state["in"]
                rs_out = rs_state["out"]
                ps = {
                    ti: pspool.tile(
                        [128, 512], fp32, tag=f"b{par + ti}", name=f"pso{ti}"
                    )
                    for ti in range(4)
                }
                for cc2 in range(8):
                    wt = wvpool.tile(
                        [128, 2, 512], bf16, tag="wvo", bufs=8, name="wot"
                    )
                    nc.sync.dma_start(
                        wt[:],
                        wo[
                            cc2 * 256 : (cc2 + 1) * 256, m0 : m0 + 512
                        ].rearrange("(a p) m -> p a m", p=128),
                    )
                    for a in range(2):
                        cc = cc2 * 2 + a
                        for ti in range(4):
                            t128 = tg * 4 + ti
                            nc.tensor.matmul(
                                ps[ti][:],
                                oT_sb[:, cc, t128 * 128 : (t128 + 1) * 128],
                                wt[:, a, :],
                                start=(cc == 0),
                                stop=False,
                            )
                    yield
                for ti in range(4):
                    nc.tensor.matmul(
                        ps[ti][:],
                        ones_row_bf_sb[:],
                        bo_sb[0:1, m0 : m0 + 512],
                        start=False,
                        stop=True,
                    )
                for ti in range(4):
                    po = outpool.tile(
                        [128, 512], bf16, tag="po", bufs=4, name="po"
                    )
                    if ti % 2 == 0:
                        nc.scalar.activation(po[:], ps[ti][:], Act.Copy)
                    else:
                        nc.vector.tensor_copy(po[:], ps[ti][:])
                    nc.sync.dma_start(
                        rs_in[ti * 128 : (ti + 1) * 128, mcol : mcol + 512],
                        po[:],
                    )
                if split or mh8 % 2 == 1:
                    nc.gpsimd.collective_compute(
                        "ReduceScatter",
                        mybir.AluOpType.add,
                        replica_groups=RG,
                        ins=[rs_in[:].opt()],
                        outs=[rs_out[:].opt()],
                    )
                    fins.append((mh8, tg, rs_out, split))
                yield

            def outproj_gen(tg, alternate, split):
                for mh8 in range(8):
                    par = (mh8 % 2) * 4 if alternate else 4
                    yield from outproj_chunk(mh8, tg, par, split)

            def zip_gens(ga, gb, na, nb):
                done_a = done_b = False
                while not (done_a and done_b):
                    for _ in range(na):
                        if not done_a:
                            try:
                                next(ga)
                            except StopIteration:
                                done_a = True
                    for _ in range(nb):
                        if not done_b:
                            try:
                                next(gb)
                            except StopIteration:
                                done_b = True

            # V block fh2=0 solo (attention heads 0-3 need it), then zip
            # the rest of V with qp=0 attention for heads 0-11.
            vgen = v_pass_gen()
            for _ in range(48):
                next(vgen)
            zip_gens(attn_gen(0, range(12)), vgen, 1, 1)
            # heads 12-15 qp0 must FULLY precede the tg=0 chunks in
            # program order (Tile dependencies are program-order RAW;
            # a read emitted before its writer reads stale data)
            for _ in attn_gen(0, range(12, 16)):
                pass
            # qp=1 attention zipped with tg=0 out-projection
            zip_gens(attn_gen(1, range(16)), outproj_gen(0, False, False), 2, 1)
            # tg=1 out-projection: bank parity + per-mh8 split RS
            for _ in outproj_gen(1, True, True):
                pass

            # Deferred output copies (plain bf16) on the GPSIMD queue: an
            # RS-gated fin there can only delay the next collective
            # trigger, which waits on the serial CC stream anyway. On ACT/
            # Sync queues a hoisted fin head-of-line-blocks compute.
            for mh8, tg, rs_out, split in fins:
                for dh in range(2):
                    if split:
                        nc.gpsimd.dma_start(
                            out[mh8 // 2, tg, dh, :,
                                (mh8 % 2) * 512 : (mh8 % 2) * 512 + 512],
                            rs_out[dh * 128 : (dh + 1) * 128, :],
                        )
                    else:
                        nc.gpsimd.dma_start(
                            out[mh8 // 2, tg, dh, :, :],
                            rs_out[dh * 128 : (dh + 1) * 128, :],
                        )

    nc.finalize()
    return nc


def _prep_shards(x, attn_bias, wq_kernel, wq_bias, wk_kernel, wk_bias,
                 wv_kernel, wv_bias, wo_kernel, wo_bias):
    """Host-side shard prep. Returns in_maps (list of 8 dicts)."""
    freqs = 1.0 / 10000.0 ** (np.arange(0, ROTARY, 2) / ROTARY)  # [16]
    pos = np.arange(MAX_POS - S, MAX_POS)  # [S]
    ang = np.outer(freqs, pos)  # [16, S]
    rotC = np.cos(ang).astype(np.float32)
    rotS = np.sin(ang).astype(np.float32)
    ones = np.ones((128, 1), dtype=BF16)
    expBT = np.exp(np.ascontiguousarray(attn_bias[0, 0].T)).astype(BF16)

    in_maps = []
    for c in range(NCORES):
        b, g = c // 2, c % 2
        hs = slice(g * HG, (g + 1) * HG)
        in_maps.append(
            {
                "xT": np.ascontiguousarray(x[b].T).astype(BF16),
                "wq": (wq_kernel[:, hs, :].reshape(D, F) * SCALE).astype(BF16),
                "wk": wk_kernel[:, hs, :].reshape(D, F).astype(BF16),
                "wv": wv_kernel[:, hs, :].reshape(D, F).astype(BF16),
                "wo": wo_kernel[hs].reshape(F, D).astype(BF16),
                "bqT": np.ascontiguousarray((wq_bias[hs] * SCALE).T).astype(
                    np.float32
                ),
                "bkT": np.ascontiguousarray(wk_bias[hs].T).astype(np.float32),
                # wv_bias folded in exactly: softmax weights sum to 1, so
                # the v-bias contributes sum_h wv_bias_h . wo_h to every
                # output row - a constant vector addable to wo_bias.
                "bo_bc": np.broadcast_to(
                    (
                        wo_bias * 0.5
                        + np.einsum(
                            "hk,hkm->m",
                            wv_bias[hs].astype(np.float64),
                            wo_kernel[hs].astype(np.float64),
                        )
                    ).reshape(1, D),
                    (128, D),
                ).astype(BF16).copy(),
                "expBT": expBT,
                "rotC": rotC.astype(BF16),
                "rotS": rotS.astype(BF16),
                "ones": ones,
                "ones_row_bf": np.ones((1, 128), dtype=BF16),
            }
        )
    return in_maps


def kernel(x, attn_bias, wq_kernel, wq_bias, wk_kernel, wk_bias,
           wv_kernel, wv_bias, wo_kernel, wo_bias, _trace=False):
    from concourse import bass_utils

    if "nc" not in _cache:
        _cache["nc"] = _build()
    nc = _cache["nc"]

    in_maps = _prep_shards(
        np.asarray(x), np.asarray(attn_bias),
        np.asarray(wq_kernel), np.asarray(wq_bias),
        np.asarray(wk_kernel), np.asarray(wk_bias),
        np.asarray(wv_kernel), np.asarray(wv_bias),
        np.asarray(wo_kernel), np.asarray(wo_bias),
    )
    res = bass_utils.run_bass_kernel_spmd(
        nc, in_maps, core_ids=list(range(NCORES)), trace=_trace
    )
    _cache["last_results"] = res

    full = np.empty((B, S, D), dtype=np.float32)
    for b in range(B):
        lo = res.results[2 * b]["out"]  # [mh4, tg, 2, 128, 1024] bf16
        hi = res.results[2 * b + 1]["out"]
        for mh4 in range(4):
            ms = slice(mh4 * 1024, (mh4 + 1) * 1024)
            for tg in range(2):
                t0 = tg * 512
                full[b, t0 : t0 + 256, ms] = lo[mh4, tg].reshape(
                    256, 1024
                ).astype(np.float32)
                full[b, t0 + 256 : t0 + 512, ms] = hi[mh4, tg].reshape(
                    256, 1024
                ).astype(np.float32)
    return full
